# revision 30
# baseline (speedup 1.0000x reference)
"""Single-head attention with per-sample padding masks, data-parallel over
batch across 8 Trainium2 NeuronCores (one batch element per core).

kernel(**inputs) takes the FULL unsharded inputs (as produced by the
problem's setup_inputs) and returns the FULL [B, N, D] float32 output.

Device program per core (SPMD, no collectives), S^T ("transposed scores")
formulation with residual-compensated fp8 matmuls:

  Every fp8 DoubleRow matmul runs at 0.5 PE cycles/column (2x bf16).
  A bf16-accuracy product a@b is computed as three fp8 terms
      a8@b8 + a8@br + ar@b8        (a8=fp8(a), ar=fp8(a-a8))
  fp8xfp8 products are exact in the fp32 PSUM, so the only error is the
  dropped ar@br term (~0.4% -- bf16 level) at 1.5 cycles/col vs bf16's 2.

  qT = (Wq'.T @ x)/64 + bq        [E, N]  (W' = 64 W staged as fp8 pair)
  kT = likewise;  v = (x.T @ Wv')/64      [N, D]
  q8/qr, k8/kr quantized on-device (Pool + DVE), vfull stays bf16.
  ST[j, i] = kT.T @ qT            3-term fp8 DR, [128 j, 512 i] blocks
  AT[j, i] = exp(s*ST + maskb_j)  maskb_j = 0 valid / -1e9 padded key ->
                                  exp -> 0. Mask rides the ACT bias.
  out[i, :] = (AT.T @ v) * valid_i/rowsum_i + colsum(v)/N * (1-valid_i)
      rowsum_i = AT.T @ ones (PE);  padded queries get mean(v) over all
      N rows, matching the reference's all-masked-row softmax.
"""

import math
import sys
from contextlib import ExitStack

import numpy as np

sys.path.insert(0, "/opt/trn_rl_repo")

import concourse.bass as bass  # noqa: E402
import concourse.mybir as mybir  # noqa: E402
import concourse.tile as tile  # noqa: E402
from concourse import bacc  # noqa: E402

P = 128
B, N, D = 8, 2048, 512
FB = 512  # psum free-dim block (one bank)
MASK_VAL = -1.0e9
# Weights pre-scaled into fp8 normal range.  32 (not 64): q' = WSCALE*q must
# stay below fp8 e4m3 max 240 -- |q| ~ N(0, 0.58), 240/32 = 7.5 is ~11 sigma,
# while 240/64 = 3.75 is reachable and one overflow -> inf -> NaN rows.
WSCALE = 32.0


def build_attention_nc(n=N, d=D, debug=False):
    """Build the one-core Bass program. Returns the compiled Bacc module."""
    f32 = mybir.dt.float32
    bf16 = mybir.dt.bfloat16
    fp8 = mybir.dt.float8e4
    DR = mybir.MatmulPerfMode.DoubleRow
    ec_n = d // P  # embedding chunks (contraction over E and D)
    nt = n // P  # 128-row seq tiles (key tiles jt / query chunks it)
    nb = n // FB  # 512-col seq blocks (query blocks ib)
    s = 1.0 / math.sqrt(d)

    nc = bacc.Bacc(None, target_bir_lowering=False, debug=debug)

    x8_d = nc.declare_dram_parameter("x8", [d, n], fp8, isOutput=False)
    xr_d = nc.declare_dram_parameter("xr", [d, n], fp8, isOutput=False)
    w8_ds, wr_ds = {}, {}
    for wn in ("wq", "wk", "wv"):
        w8_ds[wn] = nc.declare_dram_parameter(wn + "8", [d, d], fp8, isOutput=False)
        wr_ds[wn] = nc.declare_dram_parameter(wn + "r", [d, d], fp8, isOutput=False)
    bq_d = nc.declare_dram_parameter("bq", [d], f32, isOutput=False)
    bk_d = nc.declare_dram_parameter("bk", [d], f32, isOutput=False)
    maskb_d = nc.declare_dram_parameter("maskb", [P, nt], f32, isOutput=False)
    avalid_d = nc.declare_dram_parameter("avalid", [P, nt], f32, isOutput=False)
    bsel_d = nc.declare_dram_parameter("bsel", [P, nt], f32, isOutput=False)
    out_d = nc.declare_dram_parameter("out", [n, d], f32, isOutput=True)

    Ident = mybir.ActivationFunctionType.Identity
    Exp = mybir.ActivationFunctionType.Exp
    Add = mybir.AluOpType.add
    Mult = mybir.AluOpType.mult
    Sub = mybir.AluOpType.subtract

    with tile.TileContext(nc) as tc, ExitStack() as ctx:
        const = ctx.enter_context(tc.tile_pool(name="const", bufs=1))
        big = ctx.enter_context(tc.tile_pool(name="big", bufs=1))
        work = ctx.enter_context(tc.tile_pool(name="work", bufs=3))
        small = ctx.enter_context(tc.tile_pool(name="small", bufs=4))
        psum_s = ctx.enter_context(tc.tile_pool(name="psum_s", bufs=4, space="PSUM"))
        psum_av = ctx.enter_context(tc.tile_pool(name="psum_av", bufs=2, space="PSUM"))
        psum_rs = ctx.enter_context(tc.tile_pool(name="psum_rs", bufs=2, space="PSUM"))

        # ---- constants / parameters into SBUF ----
        ones_col = const.tile([P, 1], bf16)
        nc.vector.memset(ones_col, 1.0)
        ones_row = const.tile([1, P], bf16)
        nc.vector.memset(ones_row, 1.0)
        bq_sb = const.tile([P, ec_n], f32)
        bk_sb = const.tile([P, ec_n], f32)
        maskb_sb = const.tile([P, nt], f32)
        avalid_sb = const.tile([P, nt], f32)
        bsel_sb = const.tile([P, nt], f32)

        x8_sb = big.tile([P, ec_n, n], fp8)
        xr_sb = big.tile([P, ec_n, n], fp8)
        w8_sb = {wn: big.tile([P, ec_n, d], fp8, name=wn + "8_sb") for wn in ("wq", "wk", "wv")}
        wr_sb = {wn: big.tile([P, ec_n, d], fp8, name=wn + "r_sb") for wn in ("wq", "wk", "wv")}
        # coalesced DMAs (one per tensor / column-block), spread over the 3
        # DMA-capable queues so the first projection unit starts after ~2us
        def _re(dram, cols=None):
            ap = dram.ap()
            if cols is not None:
                ap = ap[:, cols[0] : cols[1]]
            return ap.rearrange("(c p) n -> p c n", p=P)

        nc.scalar.dma_start(out=w8_sb["wq"], in_=_re(w8_ds["wq"]))
        nc.scalar.dma_start(out=wr_sb["wq"], in_=_re(wr_ds["wq"]))
        for ib in range(nb):
            nc.sync.dma_start(
                out=x8_sb[:, :, ib * FB : (ib + 1) * FB],
                in_=_re(x8_d, (ib * FB, (ib + 1) * FB)),
            )
            nc.gpsimd.dma_start(
                out=xr_sb[:, :, ib * FB : (ib + 1) * FB],
                in_=_re(xr_d, (ib * FB, (ib + 1) * FB)),
            )
            if ib == 0:
                nc.scalar.dma_start(out=w8_sb["wk"], in_=_re(w8_ds["wk"]))
                nc.scalar.dma_start(out=wr_sb["wk"], in_=_re(wr_ds["wk"]))
                nc.gpsimd.dma_start(
                    out=bq_sb, in_=bq_d.ap().rearrange("(c p) -> p c", p=P)
                )
                nc.gpsimd.dma_start(
                    out=bk_sb, in_=bk_d.ap().rearrange("(c p) -> p c", p=P)
                )
            elif ib == 1:
                nc.scalar.dma_start(out=w8_sb["wv"], in_=_re(w8_ds["wv"]))
                nc.scalar.dma_start(out=wr_sb["wv"], in_=_re(wr_ds["wv"]))
                nc.gpsimd.dma_start(out=maskb_sb, in_=maskb_d[:, :])
            elif ib == 2:
                nc.gpsimd.dma_start(out=avalid_sb, in_=avalid_d[:, :])
                nc.gpsimd.dma_start(out=bsel_sb, in_=bsel_d[:, :])

        q8_sb = big.tile([P, ec_n, n], fp8)
        qr_sb = big.tile([P, ec_n, n], fp8)
        k8_sb = big.tile([P, ec_n, n], fp8)
        kr_sb = big.tile([P, ec_n, n], fp8)
        v_sb = big.tile([P, nt, d], bf16)
        v8_sb = big.tile([P, nt, d], fp8)
        vr_sb = big.tile([P, nt, d], fp8)
        a8_sb = big.tile([P, nt, n], fp8)
        ar_sb = big.tile([P, nt, n], fp8)
        mrep_sb = big.tile([P, d], f32)
        msum_row = big.tile([1, d], bf16)
        ones2_col = const.tile([P, 2, 1], fp8)
        nc.vector.memset(ones2_col, 1.0)

        def mm3(ps, a8, ar, b8, br, asl, bsl):
            """psum += a@b as a8@b8 + a8@br + ar@b8 (fp8 DoubleRow terms).
            asl/bsl: lambdas slicing [P, ec-pair, cols] views."""
            terms = [(a8, b8), (a8, br), (ar, b8)]
            nmm = len(terms) * (ec_n // 2)
            i = 0
            for ta, tb in terms:
                for ecp in range(0, ec_n, 2):
                    nc.tensor.matmul(
                        ps,
                        lhsT=asl(ta, ecp),
                        rhs=bsl(tb, ecp),
                        start=(i == 0),
                        stop=(i == nmm - 1),
                        perf_mode=DR,
                    )
                    i += 1

        # ---- Q/K projections -> fp8 pairs; V projection -> bf16 ----
        # The WSCALE on W is never divided out on-device: q', k', v' carry a
        # 64x factor that cancels in the exp scale (s/WSCALE^2) and in the
        # output normalization (avalid/bsel staged pre-divided by WSCALE).
        # q8 (ACT): fp8(ps + 64 bq) straight from PSUM; qr (DVE): one
        # scalar_tensor_tensor (ps + 64 bq) - q8.
        def emit_qkproj(ib):
            for wn, b_sb, o8, orr in (
                ("wq", bq_sb, q8_sb, qr_sb),
                ("wk", bk_sb, k8_sb, kr_sb),
            ):
                for ec in range(ec_n):
                    ps = psum_s.tile([P, FB], f32, tag="ps")
                    mm3(
                        ps,
                        w8_sb[wn],
                        wr_sb[wn],
                        x8_sb,
                        xr_sb,
                        lambda t, ecp, ec=ec: t[:, ecp : ecp + 2, ec * P : (ec + 1) * P],
                        lambda t, ecp, ib=ib: t[:, ecp : ecp + 2, ib * FB : (ib + 1) * FB],
                    )
                    sl = (slice(None), ec, slice(ib * FB, (ib + 1) * FB))
                    nc.scalar.activation(
                        o8[sl], ps, Ident, bias=b_sb[:, ec : ec + 1], scale=1.0
                    )
                    nc.vector.scalar_tensor_tensor(
                        out=orr[sl],
                        in0=ps,
                        scalar=b_sb[:, ec : ec + 1],
                        in1=o8[sl],
                        op0=Add,
                        op1=Sub,
                    )

        def emit_vproj(jt):
            ps = psum_s.tile([P, d], f32, tag="ps")
            mm3(
                ps,
                x8_sb,
                xr_sb,
                w8_sb["wv"],
                wr_sb["wv"],
                lambda t, ecp, jt=jt: t[:, ecp : ecp + 2, jt * P : (jt + 1) * P],
                lambda t, ecp: t[:, ecp : ecp + 2, :],
            )
            nc.scalar.activation(v_sb[:, jt, :], ps, Ident, bias=0.0, scale=1.0)
            nc.gpsimd.tensor_copy(v8_sb[:, jt, :], v_sb[:, jt, :])
            nc.vector.tensor_tensor(
                out=vr_sb[:, jt, :], in0=v_sb[:, jt, :], in1=v8_sb[:, jt, :], op=Sub
            )

        # interleave Q/K (DVE-heavy consumers) with V (Pool-heavy consumers)
        # so no single consumer engine gates the projection superphase
        for ib in range(nb):
            emit_qkproj(ib)
            for jt in range(ib * 4, ib * 4 + 4):
                emit_vproj(jt)

        # ---- column-sum of v (for padded-query rows: mean = colsum/N);
        # emitted between scores blocks so the PE never waits on it ----
        def emit_mean_colsum():
            pm = psum_rs.tile([1, d], f32, tag="prs")
            for jt in range(nt):
                nc.tensor.matmul(
                    pm,
                    lhsT=ones_col,
                    rhs=v_sb[:, jt, :],
                    start=(jt == 0),
                    stop=(jt == nt - 1),
                )
            nc.vector.tensor_copy(msum_row, pm)

        def emit_mean_rep():
            pr = psum_rs.tile([P, d], f32, tag="prs")
            nc.tensor.matmul(pr, lhsT=ones_row, rhs=msum_row, start=True, stop=True)
            nc.scalar.activation(mrep_sb, pr, Ident, bias=0.0, scale=1.0)

        # ---- attention ----
        def emit_scores(ib):
            for jt in range(nt):
                ps = psum_s.tile([P, FB], f32, tag="ps")
                mm3(
                    ps,
                    k8_sb,
                    kr_sb,
                    q8_sb,
                    qr_sb,
                    lambda t, ecp, jt=jt: t[:, ecp : ecp + 2, jt * P : (jt + 1) * P],
                    lambda t, ecp, ib=ib: t[:, ecp : ecp + 2, ib * FB : (ib + 1) * FB],
                )
                ab = work.tile([P, FB], bf16)
                nc.scalar.activation(
                    ab,
                    ps,
                    Exp,
                    bias=maskb_sb[:, jt : jt + 1],
                    scale=s / (WSCALE * WSCALE),
                )
                sl = (slice(None), jt, slice(ib * FB, (ib + 1) * FB))
                nc.gpsimd.tensor_copy(a8_sb[sl], ab)
                nc.vector.tensor_tensor(
                    out=ar_sb[sl], in0=ab, in1=a8_sb[sl], op=Sub
                )

        def emit_av(ib):
            for it in range(ib * 4, ib * 4 + 4):
                pav = psum_av.tile([P, d], f32, tag="pav")
                prs = psum_rs.tile([P, 1], f32, tag="prs")
                av_terms = [(a8_sb, v8_sb), (a8_sb, vr_sb), (ar_sb, v8_sb)]
                nmm = len(av_terms) * (nt // 2)
                i = 0
                for ta, tv in av_terms:
                    for jtp in range(0, nt, 2):
                        nc.tensor.matmul(
                            pav,
                            lhsT=ta[:, jtp : jtp + 2, it * P : (it + 1) * P],
                            rhs=tv[:, jtp : jtp + 2, :],
                            start=(i == 0),
                            stop=(i == nmm - 1),
                            perf_mode=DR,
                        )
                        i += 1
                i = 0
                for ta in (a8_sb, ar_sb):
                    for jtp in range(0, nt, 2):
                        nc.tensor.matmul(
                            prs,
                            lhsT=ta[:, jtp : jtp + 2, it * P : (it + 1) * P],
                            rhs=ones2_col,
                            start=(i == 0),
                            stop=(i == 2 * (nt // 2) - 1),
                            perf_mode=DR,
                        )
                        i += 1
                rinv = small.tile([P, 1], f32)
                nc.vector.reciprocal(rinv, prs)
                a_eff = small.tile([P, 1], f32)
                nc.vector.tensor_mul(a_eff, rinv, avalid_sb[:, it : it + 1])
                tmp2 = work.tile([P, d], f32)
                nc.scalar.activation(
                    tmp2, mrep_sb, Ident, bias=0.0, scale=bsel_sb[:, it : it + 1]
                )
                outt = work.tile([P, d], f32)
                nc.vector.scalar_tensor_tensor(
                    out=outt,
                    in0=pav,
                    scalar=a_eff,
                    in1=tmp2,
                    op0=Mult,
                    op1=Add,
                )
                nc.sync.dma_start(out=out_d[it * P : (it + 1) * P, :], in_=outt)

        # software-pipelined emission: scores(ib+1) runs on PE while the
        # Activation engine finishes exp(ib), so AV(ib) never stalls the PE;
        # the mean chain hides inside the scores blocks
        emit_scores(0)
        emit_mean_colsum()
        emit_scores(1)
        emit_mean_rep()
        emit_av(0)
        emit_scores(2)
        emit_av(1)
        emit_scores(3)
        emit_av(2)
        emit_av(3)

    nc.compile()
    return nc


def _fp8_pair(a, npdt):
    """Return (fp8(a), fp8(a - fp8(a))) as numpy arrays of dtype npdt."""
    a = np.asarray(a, np.float32)
    a8 = a.astype(npdt)
    ar = (a - a8.astype(np.float32)).astype(npdt)
    return a8, ar


def make_in_maps(x, event_lengths, Wq, bq, Wk, bk, Wv, bv, n=N, d=D):
    """Host-side sharding + marshaling: one batch element per core."""
    npdt = mybir.dt.np(mybir.dt.float8e4)
    x = np.asarray(x, dtype=np.float32)
    lens = np.asarray(event_lengths).astype(np.int64)
    ws = {}
    for wn, W in (("wq", Wq), ("wk", Wk), ("wv", Wv)):
        wT = np.ascontiguousarray(np.asarray(W, np.float32).T) * WSCALE
        ws[wn + "8"], ws[wn + "r"] = _fp8_pair(wT, npdt)
    # biases enter the PSUM which carries a WSCALE factor; valid/bsel are
    # staged pre-divided by WSCALE so the 64x on v' cancels at the output
    bq = np.asarray(bq, np.float32) * np.float32(WSCALE)
    bk = np.asarray(bk, np.float32) * np.float32(WSCALE)
    idx = np.arange(n)
    valid2d = np.ascontiguousarray(
        (idx.reshape(n // P, P).T[None, :, :] < lens[:, None, None])
    ).astype(np.float32)  # [B, P, nt] : valid2d[b, p, t] = (t*128+p < L_b)
    in_maps = []
    for b in range(x.shape[0]):
        va = valid2d[b]
        x8, xr = _fp8_pair(np.ascontiguousarray(x[b].T), npdt)
        in_maps.append(
            {
                "x8": x8,
                "xr": xr,
                **ws,
                "bq": bq,
                "bk": bk,
                "maskb": (1.0 - va) * MASK_VAL,
                "avalid": va / np.float32(WSCALE),
                "bsel": (1.0 - va) / np.float32(n * WSCALE),
            }
        )
    return in_maps


_NC_CACHE = {}


def kernel(x, event_lengths, Wq, bq, Wk, bk, Wv, bv):
    from concourse.bass_utils import run_bass_kernel_spmd

    if "nc" not in _NC_CACHE:
        _NC_CACHE["nc"] = build_attention_nc()
    nc = _NC_CACHE["nc"]
    in_maps = make_in_maps(x, event_lengths, Wq, bq, Wk, bk, Wv, bv)
    res = run_bass_kernel_spmd(nc, in_maps, core_ids=list(range(B)))
    out = np.stack([np.asarray(r["out"], np.float32) for r in res.results], axis=0)
    return out
